# revision 32
# baseline (speedup 1.0000x reference)
"""DCNv2 (deformable conv + BN + ReLU) Trainium2 Bass kernel, 8-core SPMD.

Sharding: core c owns sample b=c//4, output rows [24*(c%4), 24*(c%4)+24),
processed as 6 T-tiles of 4 rows (384 positions each).

Position relabeling: within a T-tile, conv column col = t*96+w is assigned
pipeline position l = q*128 + pp*16 + r where col = r*24 + q*8 + pp.
This makes the gather-index repack DMA contiguous in 48B runs:
  idxG[r, k*24 + c] = idx16[k, r*24 + c]   (c = col%24)
and the gather consumes idxG[16, 216] in n = s*16+r order with
n = ((k%3)*3+q)*128 + (pp*16+r), exactly the corner-matmul layout.

Per T (software-pipelined; tile pools give cross-T overlap):
  1. offset conv on PE -> pom[96, 384] (y/x/m channel groups at
     partitions 0/32/64 so engine slices are 32-aligned)
  2. coefficients on DVE (conv layout): magic-round floor, clamp,
     gather index; sigmoid mask on ACT; bilinear products on Pool,
     written in l-order for contiguous transposes
  3. PE transposes [9,128]->[128,9] put a-coeffs/mask pos-major
  4. idx repack via DRAM roundtrip (48B-run strided write, contiguous
     readback), then 3x dma_gather of 2KB 4-corner rows (bf16)
  5. corner-sum on PE, third-major so third t needs only gather t:
     S[ch,(k,cf),pos] += G^T @ (ident * wy*wx * m) per corner
     (12-diag dg tiles built on DVE with dual-scalar mults)
  6. per-q main GEMM on PE; BN sums fused into psum->sbuf copy via
     accum_out; Square pass accumulates sum-of-squares
  7. tail: BN stats AllReduce (8 cores), scale/shift/ReLU on ACT
     (sqrt act-table preloaded during the loop), chunked bf16 stores.
PE warm-up matmuls at t=0 ramp the p-state during input loads.
"""

import numpy as np
import ml_dtypes

BF16 = ml_dtypes.bfloat16
B, CI, CO, H, W = 2, 256, 256, 96, 96
NCORES = 8
RB = 24                      # output rows per core
NPOS = RB * W                # 2304 positions per core
PADG = 8                     # gather-table pad on each side
GRID = H + 2 * PADG          # 112
NROWS = GRID * GRID          # 12544 table rows
NTOT = float(B * H * W)      # BN count
EPS = 1e-5
MAGIC = 8388608.0            # 2^23 for round-to-floor trick

KY9 = np.repeat(np.arange(3), 3).astype(np.float32)
KX9 = np.tile(np.arange(3), 3).astype(np.float32)

_CACHE = {}


def _build_program():
    import concourse.bass as bass
    from concourse import bacc, tile, mybir

    ds = bass.ds
    f32 = mybir.dt.float32
    bf16 = mybir.dt.bfloat16
    i16 = mybir.dt.int16
    Alu = mybir.AluOpType
    Act = mybir.ActivationFunctionType

    nc = bacc.Bacc("TRN2", target_bir_lowering=False, debug=False,
                   num_devices=NCORES, dynamic_dma_scratch_size=32768)

    tab_d = nc.dram_tensor("tab", [NROWS, 1024], bf16, kind="ExternalInput")
    slab_d = nc.dram_tensor("slab", [128, 2, RB + 2, W + 2], bf16,
                            kind="ExternalInput")
    woff_d = nc.dram_tensor("woff", [128, 2, 9, 96], bf16,
                            kind="ExternalInput")
    pypx_d = nc.dram_tensor("pypx", [96, 6, 384], f32, kind="ExternalInput")
    wdcn_d = nc.dram_tensor("wdcn", [128, 18, 2, 128], bf16,
                            kind="ExternalInput")
    ident_d = nc.dram_tensor("ident", [128, 128], bf16, kind="ExternalInput")
    identf_d = nc.dram_tensor("identf", [128, 128], f32, kind="ExternalInput")
    gb_d = nc.dram_tensor("gb", [128, 2, 3], f32, kind="ExternalInput")
    out_d = nc.dram_tensor("out", [2, 128, NPOS], bf16, kind="ExternalOutput")

    with tile.TileContext(nc) as tc:
        with (
            tc.tile_pool(name="cst", bufs=1) as cst,
            tc.tile_pool(name="sb", bufs=1) as sb,
            tc.tile_pool(name="cf", bufs=2) as cf,
            tc.tile_pool(name="gpool", bufs=3) as gpool,
            tc.tile_pool(name="apool", bufs=2) as apool,
            tc.tile_pool(name="dpool", bufs=6) as dpool,
            tc.tile_pool(name="spool", bufs=2) as spool,
            tc.tile_pool(name="opool", bufs=3) as opool,
            tc.tile_pool(name="ps_om", bufs=1, space="PSUM") as ps_om,
            tc.tile_pool(name="ps_t", bufs=1, space="PSUM") as ps_t,
            tc.tile_pool(name="ps_s", bufs=2, space="PSUM") as ps_s,
            tc.tile_pool(name="ps_o", bufs=2, space="PSUM") as ps_o,
            tc.tile_pool(name="dram", bufs=1, space="DRAM") as dram,
        ):
            # ---------- PE warm-up: ramp p-state during input loads ----
            wident = cst.tile([128, 128], bf16)
            nc.vector.memset(wident[:], 0)
            wps = ps_om.tile([96, 384], f32, tag="pom")
            for _ in range(80):
                nc.tensor.matmul(wps[:, 0:128], wident[:, 0:96],
                                 wident[:])

            # ---------- persistent tiles ----------
            slab = cst.tile([128, 2, RB + 2, W + 2], bf16)
            nc.sync.dma_start(slab[:], slab_d[:])
            woff = cst.tile([128, 2, 9, 96], bf16)
            nc.sync.dma_start(woff[:], woff_d[:])
            pypx = cst.tile([96, 6, 384], f32)
            nc.sync.dma_start(pypx[:], pypx_d[:])
            wdcn = cst.tile([128, 18, 2, 128], bf16)
            nc.sync.dma_start(wdcn[:], wdcn_d[:])
            ident = cst.tile([128, 128], bf16)
            nc.sync.dma_start(ident[:], ident_d[:])
            identf = cst.tile([128, 128], f32)
            nc.sync.dma_start(identf[:], identf_d[:])
            gb = cst.tile([128, 2, 3], f32)
            nc.sync.dma_start(gb[:], gb_d[:])

            idxG = sb.tile([128, 2, 216], i16)
            nc.vector.memset(idxG[:], 0)
            d4 = dram.tile([2, 3456], i16)
            out_sb = sb.tile([128, 2, NPOS], bf16)
            SU = sb.tile([128, 2, 18], f32)  # per-(T,q) BN sums
            SQ = sb.tile([128, 2, 6], f32)   # per-T BN sum-of-squares

            ident_b = ident[:].rearrange("p (one n) -> p one n", one=1) \
                .broadcast_to([128, 36, 128])

            def conv(T):
                pom = ps_om.tile([96, 384], f32, tag="pom")
                first = True
                for ct in range(2):
                    for k in range(9):
                        ky, kx = int(KY9[k]), int(KX9[k])
                        rhs = slab[:, ct, T * 4 + ky:T * 4 + ky + 4,
                                   kx:kx + 96]
                        nc.tensor.matmul(pom[:], woff[:, ct, k, :], rhs,
                                         start=first,
                                         stop=(ct == 1 and k == 8))
                        first = False
                return pom

            def idx_coeffs(T, pom):
                # coefficients in conv layout [<=36 part, 384 col]
                opp = cf.tile([96, 384], f32, tag="opp")
                nc.vector.tensor_tensor(opp[:], pom[:], pypx[:, T], Alu.add)
                msk = cf.tile([9, 384], f32, tag="msk")
                # msk stored in l-order so its transpose input is contiguous
                nc.scalar.activation(
                    msk[:].rearrange("k (q2 pp r) -> k r q2 pp", q2=3, pp=8),
                    opp[64:73].rearrange("k (r q2 pp) -> k r q2 pp",
                                         r=16, q2=3),
                    Act.Sigmoid)
                iyx = cf.tile([64, 384], f32, tag="iyx")
                # floor via round(x - 0.5); exact-int x floors one low
                # (harmless by bilinear continuity). y rows 0:9, x rows
                # 32:41; in-between rows are well-defined junk.
                nc.vector.tensor_scalar(iyx[:], opp[0:64], MAGIC - 0.5,
                                        -MAGIC, Alu.add, Alu.add)
                fyx = cf.tile([64, 384], f32, tag="fyx")
                nc.vector.tensor_tensor(fyx[:], opp[0:64], iyx[:],
                                        Alu.subtract)
                nc.vector.tensor_scalar(iyx[:], iyx[:], 8.0, 118.0, Alu.max,
                                        Alu.min)
                ix9 = cf.tile([9, 384], f32, tag="ix9")
                nc.vector.tensor_copy(ix9[:], iyx[32:41])
                idxf = cf.tile([9, 384], f32, tag="idxf")
                nc.vector.tensor_scalar(idxf[:], iyx[0:9], float(GRID),
                                        -904.0, Alu.mult, Alu.add)
                nc.vector.tensor_tensor(idxf[:], idxf[:], ix9[:], Alu.add)
                idx16 = cf.tile([9, 384], i16, tag="idx16")
                nc.vector.tensor_copy(idx16[:], idxf[:])

                # idx repack via DRAM (contiguous 48B runs), then gathers
                slot = T % 2
                nc.sync.dma_start(
                    d4[slot].rearrange("(r k c) -> k r c", r=16, k=9),
                    idx16[:].rearrange("k (r c) -> k r c", r=16))
                nc.sync.dma_start(idxG[0:16, slot, :],
                                  d4[slot].rearrange("(r s) -> r s", r=16))
                gt = []
                for kc in range(3):
                    g = gpool.tile([128, 9, 1024], bf16, tag="g")
                    nc.gpsimd.dma_gather(
                        g[:], tab_d[:], idxG[:, slot, kc * 72:(kc + 1) * 72],
                        num_idxs=1152, num_idxs_reg=1152, elem_size=1024)
                    gt.append(g)
                return msk, fyx, gt

            def a_coeffs(T, fyx):
                # tensor_tensor operands must share a base partition, so
                # copy the x rows (base 32) down to base-0 tiles first.
                # T=0 runs on DVE so Pool can desc-gen the first gathers.
                eng = nc.vector if T == 0 else nc.gpsimd
                wyx0 = cf.tile([64, 384], f32, tag="wyx0")
                eng.tensor_scalar(wyx0[:], fyx[:], -1.0, 1.0, Alu.mult,
                                  Alu.add)
                wx9 = cf.tile([9, 384], f32, tag="wx9")
                eng.tensor_copy(wx9[:], wyx0[32:41])
                fx9 = cf.tile([9, 384], f32, tag="fx9")
                eng.tensor_copy(fx9[:], fyx[32:41])
                # products written in l-order (out AP strided) so the
                # per-q transpose inputs are contiguous [9, 128] slices
                aFj = cf.tile([9, 4, 384], f32, tag="aFj")

                def lv(ap):
                    return ap.rearrange("k (r q2 pp) -> k r q2 pp",
                                        r=16, q2=3)

                def lo(j):
                    return aFj[:, j, :].rearrange(
                        "k (q2 pp r) -> k r q2 pp", q2=3, pp=8)

                eng.tensor_tensor(lo(0), lv(wyx0[0:9]), lv(wx9[:]),
                                  Alu.mult)
                eng.tensor_tensor(lo(1), lv(wyx0[0:9]), lv(fx9[:]),
                                  Alu.mult)
                eng.tensor_tensor(lo(2), lv(fyx[0:9]), lv(wx9[:]),
                                  Alu.mult)
                eng.tensor_tensor(lo(3), lv(fyx[0:9]), lv(fx9[:]),
                                  Alu.mult)
                return aFj

            def a_transpose(T, aFj, msk):
                # PE transposes per (q, j): [9, (pp,r)=128] -> [128, 9]
                # packed at cols j*9+k; mask -> cols 36:45
                a_ps = ps_t.tile([128, 3, 48], f32, tag="aps")
                for q in range(3):
                    for j in range(4):
                        nc.tensor.matmul(a_ps[:, q, j * 9:(j + 1) * 9],
                                         aFj[:, j,
                                             q * 128:(q + 1) * 128],
                                         identf[0:9, 0:9], is_transpose=True)
                    nc.tensor.matmul(a_ps[:, q, 36:45],
                                     msk[:, q * 128:(q + 1) * 128],
                                     identf[0:9, 0:9], is_transpose=True)
                a_pos = apool.tile([128, 3, 48], f32, tag="apos")
                nc.vector.tensor_copy(a_pos[:, :, 0:45], a_ps[:, :, 0:45])
                return a_pos

            def dg_one(T, t, q, a_pos):
                # 12 diags for (third t, q-block): k in {3t..3t+2} x 4 corners
                dgs = dpool.tile([128, 12, 128], bf16, tag="dg")
                for kk in range(3):
                    k = t * 3 + kk
                    for j in range(4):
                        nc.vector.tensor_scalar(
                            dgs[:, kk * 4 + j, :], ident[:],
                            a_pos[:, q, j * 9 + k:j * 9 + k + 1],
                            a_pos[:, q, 36 + k:37 + k],
                            Alu.mult, Alu.mult)
                return dgs

            def corner_tq(T, t, q, g, dgs, s_sb):
                # third t only reads gather tile t
                pss = ps_s.tile([128, 6, 128], f32, tag="pss")
                for chl in range(6):
                    k, cfh = t * 3 + chl // 2, chl % 2
                    slot9 = (k % 3) * 3 + q
                    for j in range(4):
                        lhsT = g[:, slot9, j * 256 + cfh * 128:
                                 j * 256 + cfh * 128 + 128]
                        nc.tensor.matmul(pss[:, chl, :], lhsT,
                                         dgs[:, (chl // 2) * 4 + j, :],
                                         start=(j == 0), stop=(j == 3))
                nc.scalar.copy(s_sb[:, t * 6:t * 6 + 6,
                                    q * 128:(q + 1) * 128], pss[:])

            def gemm_q(T, q, s_sb):
                po = ps_o.tile([128, 2, 128], f32, tag="po")
                for o2 in range(2):
                    for ch in range(18):
                        nc.tensor.matmul(po[:, o2, :], wdcn[:, ch, o2, :],
                                         s_sb[:, ch, q * 128:(q + 1) * 128],
                                         start=(ch == 0), stop=(ch == 17))
                for o2 in range(2):
                    osl = out_sb[:, o2, T * 384 + q * 128:
                                 T * 384 + (q + 1) * 128]
                    nc.scalar.activation(osl, po[:, o2, :], Act.Identity,
                                         bias=gb[:, o2, 2:3],
                                         accum_out=SU[:, o2,
                                                      T * 3 + q:T * 3 + q + 1])

            def square(T):
                for o2 in range(2):
                    scrap = sb.tile([128, 384], bf16, tag="scrap")
                    nc.scalar.activation(scrap[:],
                                         out_sb[:, o2,
                                                T * 384:(T + 1) * 384],
                                         Act.Square,
                                         accum_out=SQ[:, o2, T:T + 1])

            # ---------- software-pipelined main loop ----------
            # corner loops are third-major: third t consumes only gather
            # tile t, so compute starts as soon as the first gather lands
            pom = conv(0)
            msk, fyx, gt = idx_coeffs(0, pom)
            aF = a_coeffs(0, fyx)
            a_pos = a_transpose(0, aF, msk)
            junk = sb.tile([1, 2], f32)
            for T in range(6):
                if T == 5:
                    # preload sqrt act-table after the last Sigmoid (the
                    # Square dep pins it late; Relu/Copy/Square are in the
                    # sqrt set too, so the tail needs no further switch)
                    nc.scalar.activation(junk[:, 0:1], SU[0:1, 1, 14:15],
                                         Act.Square)
                    nc.scalar.sqrt(junk[:, 1:2], junk[:, 0:1])
                s_sb = spool.tile([128, 18, 384], bf16, tag="s")
                d0 = [dg_one(T, 0, q, a_pos) for q in range(3)]
                d1 = [dg_one(T, 1, q, a_pos) for q in range(3)]
                for q in range(3):
                    corner_tq(T, 0, q, gt[0], d0[q], s_sb)
                if T < 5:
                    pom = conv(T + 1)
                    msk, fyx, gt_n = idx_coeffs(T + 1, pom)
                for q in range(3):
                    corner_tq(T, 1, q, gt[1], d1[q], s_sb)
                if T < 5:
                    aF = a_coeffs(T + 1, fyx)
                d2 = [dg_one(T, 2, 0, a_pos), dg_one(T, 2, 1, a_pos), None]
                corner_tq(T, 2, 0, gt[2], d2[0], s_sb)
                if T < 5:
                    a_posn = a_transpose(T + 1, aF, msk)
                gemm_q(T, 0, s_sb)
                d2[2] = dg_one(T, 2, 2, a_pos)
                corner_tq(T, 2, 1, gt[2], d2[1], s_sb)
                gemm_q(T, 1, s_sb)
                corner_tq(T, 2, 2, gt[2], d2[2], s_sb)
                gemm_q(T, 2, s_sb)
                square(T)
                if T < 5:
                    gt = gt_n
                    a_pos = a_posn

            # ---------- BN stats + allreduce + finish ----------
            part = sb.tile([128, 4], f32)
            for o2 in range(2):
                nc.vector.tensor_reduce(part[:, 2 * o2:2 * o2 + 1],
                                        SU[:, o2, :],
                                        mybir.AxisListType.X, Alu.add)
                nc.vector.tensor_reduce(part[:, 2 * o2 + 1:2 * o2 + 2],
                                        SQ[:, o2, :],
                                        mybir.AxisListType.X, Alu.add)
            bin_d = dram.tile([128, 4], f32)
            bout_d = dram.tile([128, 4], f32, addr_space="Shared")
            import os as _os
            nc.sync.dma_start(bin_d[:], part[:])
            if _os.environ.get("NOCC", "0") == "1":
                nc.sync.dma_start(bout_d[:], bin_d[:])
            else:
                nc.gpsimd.collective_compute(
                    "AllReduce", mybir.AluOpType.add,
                    replica_groups=[list(range(NCORES))],
                    ins=[bin_d[:].opt()], outs=[bout_d[:].opt()])
            stats = sb.tile([128, 4], f32)
            nc.sync.dma_start(stats[:], bout_d[:])
            tmp = sb.tile([128, 8], f32)
            for o2 in range(2):
                eng = nc.vector if o2 == 0 else nc.gpsimd
                mean = tmp[:, 4 * o2 + 0:4 * o2 + 1]
                var = tmp[:, 4 * o2 + 1:4 * o2 + 2]
                s_ = tmp[:, 4 * o2 + 2:4 * o2 + 3]
                t_ = tmp[:, 4 * o2 + 3:4 * o2 + 4]
                eng.tensor_scalar_mul(mean, stats[:, 2 * o2:2 * o2 + 1],
                                      1.0 / NTOT)
                eng.tensor_scalar_mul(var,
                                      stats[:, 2 * o2 + 1:2 * o2 + 2],
                                      1.0 / NTOT)
                eng.tensor_tensor(s_, mean, mean, Alu.mult)
                eng.tensor_tensor(var, var, s_, Alu.subtract)
                eng.tensor_scalar_add(var, var, EPS)
                nc.scalar.sqrt(s_, var)
                nc.vector.reciprocal(s_, s_)
                eng.tensor_tensor(s_, s_, gb[:, o2, 0:1], Alu.mult)
                eng.tensor_tensor(t_, mean, s_, Alu.mult)
                eng.tensor_scalar_mul(t_, t_, -1.0)
                eng.tensor_tensor(t_, t_, gb[:, o2, 1:2], Alu.add)
            for o2 in range(2):
                s_ = tmp[:, 4 * o2 + 2:4 * o2 + 3]
                t_ = tmp[:, 4 * o2 + 3:4 * o2 + 4]
                for hh in range(4):
                    outf = opool.tile([128, 576], bf16, tag="outf")
                    osl = out_sb[:, o2, hh * 576:(hh + 1) * 576]
                    if hh % 2 == 0:
                        nc.scalar.activation(outf[:], osl, Act.Relu,
                                             bias=t_, scale=s_)
                    else:
                        nc.vector.tensor_scalar(outf[:], osl, s_, t_,
                                                Alu.mult, Alu.add)
                        nc.vector.tensor_scalar_max(outf[:], outf[:], 0.0)
                    nc.sync.dma_start(out_d[o2, :, hh * 576:(hh + 1) * 576],
                                      outf[:])

    nc.compile()
    return nc


# position permutation: l = q*128 + pp*16 + r for col = r*24 + q*8 + pp
_COL = np.arange(384)
_LUT = (_COL % 24 // 8) * 128 + (_COL % 8) * 16 + _COL // 24  # col -> l


def _prep_inputs(x, w_off, b_off, w_dcn, b_dcn, gamma, beta):
    """Build the 8 per-core input maps (host-side sharding/layout only)."""
    x = np.asarray(x, np.float32)
    w_off = np.asarray(w_off, np.float32)
    b_off = np.asarray(b_off, np.float32)
    w_dcn = np.asarray(w_dcn, np.float32)
    b_dcn = np.asarray(b_dcn, np.float32)
    gamma = np.asarray(gamma, np.float32)
    beta = np.asarray(beta, np.float32)

    # 4-corner gather tables per sample
    P = PADG
    xp = np.zeros((B, CI, GRID + 1, GRID + 1), np.float32)
    xp[:, :, P:P + H, P:P + W] = x
    xp = xp.astype(BF16)
    tabs = []
    for b in range(B):
        t = np.empty((GRID, GRID, 4, CI), BF16)
        for j, (dy2, dx2) in enumerate([(0, 0), (0, 1), (1, 0), (1, 1)]):
            t[:, :, j, :] = np.moveaxis(
                xp[b, :, dy2:dy2 + GRID, dx2:dx2 + GRID], 0, -1)
        tabs.append(np.ascontiguousarray(t.reshape(NROWS, 1024)))

    # conv slab (1-pixel zero pad) per sample, bf16, [128, ct, 26, 98]
    xs = np.zeros((B, CI, H + 2, W + 2), np.float32)
    xs[:, :, 1:H + 1, 1:W + 1] = x
    xs = xs.astype(BF16)

    # offset-conv weights, output channels permuted to [dy*9, dx*9, m*9]
    perm = np.concatenate([np.arange(0, 17, 2), np.arange(1, 18, 2),
                           np.arange(18, 27)])
    wofp = w_off[perm]            # [27, CI, 3, 3]
    boffp = b_off[perm]
    w27 = np.ascontiguousarray(
        wofp.reshape(27, 2, 128, 3, 3).transpose(2, 1, 3, 4, 0)
        .reshape(128, 2, 9, 27)).astype(BF16)
    # out channels embedded at partition groups 0:9 (y), 32:41 (x),
    # 64:73 (m) so engine slices start at multiples of 32
    woff_h = np.zeros((128, 2, 9, 96), BF16)
    woff_h[:, :, :, 0:9] = w27[:, :, :, 0:9]
    woff_h[:, :, :, 32:41] = w27[:, :, :, 9:18]
    woff_h[:, :, :, 64:73] = w27[:, :, :, 18:27]

    # wdcn lhsT chunks: [p, ch=(k*2+cf), o2, oc] = w_dcn[o2*128+oc, cf*128+p, k]
    wd = w_dcn.reshape(CO, CI, 9)
    wdcn_h = np.ascontiguousarray(
        wd.reshape(2, 128, 2, 128, 9).transpose(3, 4, 2, 0, 1)
        .reshape(128, 9, 2, 2, 128).transpose(0, 1, 2, 3, 4)
        .reshape(128, 18, 2, 128)).astype(BF16)

    ident_h = np.eye(128, dtype=BF16)
    identf_h = np.eye(128, dtype=np.float32)
    gb_h = np.zeros((128, 2, 3), np.float32)
    for o2 in range(2):
        gb_h[:, o2, 0] = gamma[o2 * 128:(o2 + 1) * 128]
        gb_h[:, o2, 1] = beta[o2 * 128:(o2 + 1) * 128]
        gb_h[:, o2, 2] = b_dcn[o2 * 128:(o2 + 1) * 128]

    tt = np.arange(4, dtype=np.float32)   # row within T-tile
    ww = np.arange(96, dtype=np.float32)
    in_maps = []
    for c in range(NCORES):
        b, rb = c // 4, c % 4
        slab_h = np.ascontiguousarray(
            xs[b].reshape(2, 128, H + 2, W + 2)
            .transpose(1, 0, 2, 3)[:, :, rb * RB:rb * RB + RB + 2, :])
        # pypx [96, 6T, 384col]: +16 (grid offset) folded into y/x rows;
        # rows 0:9 = y base, 32:41 = x base, 64:73 = mask bias
        pypx_h = np.zeros((96, 6, 384), np.float32)
        for T in range(6):
            py = np.broadcast_to(
                rb * RB + T * 4 - 1.0 + 16.0 + tt[None, :, None]
                + KY9[:, None, None] + boffp[0:9, None, None], (9, 4, 96))
            px = (ww[None, None, :] - 1.0 + 16.0
                  + KX9[:, None, None] + boffp[9:18, None, None])
            px = np.broadcast_to(px, (9, 4, 96))
            pypx_h[0:9, T] = py.reshape(9, 384)
            pypx_h[32:41, T] = px.reshape(9, 384)
            pypx_h[64:73, T] = boffp[18:27, None]
        in_maps.append({
            "tab": tabs[b], "slab": slab_h, "woff": woff_h,
            "pypx": pypx_h, "wdcn": wdcn_h, "ident": ident_h,
            "identf": identf_h, "gb": gb_h,
        })
    return in_maps


def kernel(x, w_off, b_off, w_dcn, b_dcn, gamma, beta, _trace=False):
    import os
    if "nc" not in _CACHE:
        _CACHE["nc"] = _build_program()
    nc = _CACHE["nc"]
    in_maps = _prep_inputs(x, w_off, b_off, w_dcn, b_dcn, gamma, beta)
    results = None
    if os.environ.get("FORCE_SIM", "0") == "1":
        from concourse import bass_interp
        sim = bass_interp.MultiCoreSim(nc, NCORES)
        for c in range(NCORES):
            for name, val in in_maps[c].items():
                sim.cores[c].tensor(name)[:] = val
        sim.simulate()
        results = [{"out": np.asarray(sim.cores[c].tensor("out"))}
                   for c in range(NCORES)]
    else:
        from concourse.bass_utils import run_bass_kernel_spmd
        try:
            try:
                res = run_bass_kernel_spmd(nc, in_maps,
                                           core_ids=list(range(NCORES)),
                                           trace=_trace)
            except ModuleNotFoundError:
                res = run_bass_kernel_spmd(nc, in_maps,
                                           core_ids=list(range(NCORES)),
                                           trace=False)
            _CACHE["last"] = res
            results = res.results
        except Exception:
            # hardware path unavailable: fall back to multi-core simulator
            from concourse import bass_interp
            sim = bass_interp.MultiCoreSim(nc, NCORES)
            for c in range(NCORES):
                for name, val in in_maps[c].items():
                    sim.cores[c].tensor(name)[:] = val
            sim.simulate()
            results = [{"out": np.asarray(sim.cores[c].tensor("out"))}
                       for c in range(NCORES)]
    out = np.empty((B, CO, H, W), np.float32)
    for c in range(NCORES):
        b, rb = c // 4, c % 4
        o = results[c]["out"]  # [2, 128, NPOS]
        ot = o.reshape(CO, 6, 384)[:, :, _LUT]       # [CO, 6T, col]
        out[b, :, rb * RB:(rb + 1) * RB, :] = ot.reshape(CO, RB, W)
    return out
